# revision 12
# baseline (speedup 1.0000x reference)
"""Expert-parallel MoE kernel for Trainium2 (8 NeuronCores, 1 expert per core).

Strategy:
  - Host computes routing (top-k affinity normalization + combine weights) and
    gathers each expert's tokens; core e processes expert e's routed tokens only.
  - Mixed precision by combine weight: each expert's routed pairs are split
    into class A (the CA highest combine-weight tokens, fp16 path — exact
    centered int8 codes in fp16) and class B (the rest, lowest combine
    weights, fp8-e4m3 DoubleRow path at 2x PE throughput). Class-B error
    (~6% per pair) is diluted by its small combine weight; global rel err
    lands ~1.5e-2 (< 2e-2 gate).
  - Quantized weights are uploaded as CENTERED integer codes (q-128): fp16
    (exact) for class A, e4m3 (RNE) for class B; per-output-channel scales
    are applied on-chip AFTER the matmul.
  - Matmuls keep weights as the stationary operand; activations/intermediates
    flow as [channel_partition, token_free] tiles so gate_up -> glu -> down
    chains with zero transposes.
  - Class A folds the combine weight into the GLU epilogue; class B folds it
    after the down matmul (fp8 h must not be scaled by tiny combine weights),
    and carries a /2 on h with a x2 on the down scale to keep |h| < 240
    (TRN e4m3 overflows to inf, not saturate).
  - Host scatter-adds the per-expert outputs back to the full [T, H] output.
"""

import math
from contextlib import ExitStack

import numpy as np
import ml_dtypes

import concourse.bass as bass
import concourse.tile as tile
import concourse.mybir as mybir
from concourse import bacc
from concourse.bass_utils import run_bass_kernel_spmd

E, H, I, TOPK = 8, 4096, 1792, 2
ZP = 128.0
P = 128
KH = H // P          # 32 contraction slabs for gate_up
KI = I // P          # 14 contraction slabs for down
NJ = I // P          # 14 gate/up pair groups (each 128 gate + 128 up cols)
NG = (H // P) // 2   # 16 down output groups (each 256 out cols)

fp16 = mybir.dt.float16
fp8 = mybir.dt.float8e4
fp32 = mybir.dt.float32
DR = mybir.MatmulPerfMode.DoubleRow
E4M3 = ml_dtypes.float8_e4m3


def build_moe_nc(CA, CB, num_devices=8):
    """Per-core MoE program: class A (CA tokens, fp16) + class B (CB, fp8 DR).
    CA must be even (two psum chunks of WA=CA/2); CB <= 512, CB % 16 == 0."""
    WA = CA // 2
    tcA = 2
    assert CA % 2 == 0 and WA <= 512 and CB <= 512 and CB % 16 == 0
    C = CA + CB

    nc = bacc.Bacc("TRN2", target_bir_lowering=False, debug=False,
                   num_devices=num_devices)
    x8d = nc.dram_tensor("x8", [P, KH, CB], fp8, kind="ExternalInput").ap()
    xT = nc.dram_tensor("xT", [tcA, P, KH, WA], fp16, kind="ExternalInput").ap()
    w8gu = nc.dram_tensor("w8gu", [NJ, P, KH // 2, 2, 256], fp8,
                          kind="ExternalInput").ap()
    wgu = nc.dram_tensor("wgu", [NJ, P, KH, 256], fp16, kind="ExternalInput").ap()
    wd8 = nc.dram_tensor("wd8", [NG, P, KI // 2, 2, 256], fp8,
                         kind="ExternalInput").ap()
    wd = nc.dram_tensor("wd", [NG, P, KI, 256], fp16, kind="ExternalInput").ap()
    sgu8 = nc.dram_tensor("sgu8", [P, 2 * NJ], fp32, kind="ExternalInput").ap()
    sgu = nc.dram_tensor("sgu", [P, 2 * NJ], fp32, kind="ExternalInput").ap()
    sd8 = nc.dram_tensor("sd8", [P, 2 * NG], fp32, kind="ExternalInput").ap()
    sd = nc.dram_tensor("sd", [P, 2 * NG], fp32, kind="ExternalInput").ap()
    wcA = nc.dram_tensor("wcA", [P, CA], fp32, kind="ExternalInput").ap()
    wcB = nc.dram_tensor("wcB", [P, CB], fp32, kind="ExternalInput").ap()
    out = nc.dram_tensor("out", [P, H // P, C], fp32, kind="ExternalOutput").ap()

    with tile.TileContext(nc) as tcx, ExitStack() as ctx:
        const_pool = ctx.enter_context(tcx.tile_pool(name="const", bufs=1))
        wpool = ctx.enter_context(tcx.tile_pool(name="w", bufs=2))
        w8pool = ctx.enter_context(tcx.tile_pool(name="w8", bufs=4))
        wdpool = ctx.enter_context(tcx.tile_pool(name="wd", bufs=2))
        wd8pool = ctx.enter_context(tcx.tile_pool(name="wd8", bufs=4))
        hpool = ctx.enter_context(tcx.tile_pool(name="h", bufs=1))
        tmp_pool = ctx.enter_context(tcx.tile_pool(name="tmp", bufs=3))
        out_pool = ctx.enter_context(tcx.tile_pool(name="outp", bufs=3))
        psum_pool = ctx.enter_context(tcx.tile_pool(name="psum", bufs=8, space="PSUM"))

        x8_sb = const_pool.tile([P, KH, CB], fp8)
        xT_sb = const_pool.tile([P, tcA, KH, WA], fp16)
        h8_sb = hpool.tile([P, KI, CB], fp8)
        h_sb = hpool.tile([P, KI, CA], fp16)
        sgu8_sb = const_pool.tile([P, 2 * NJ], fp32)
        sgu_sb = const_pool.tile([P, 2 * NJ], fp32)
        sd8_sb = const_pool.tile([P, 2 * NG], fp32)
        sd_sb = const_pool.tile([P, 2 * NG], fp32)
        wcA_sb = const_pool.tile([P, CA], fp32)
        wcB_sb = const_pool.tile([P, CB], fp32)

        # Warm up the PE clock (HAM un-throttle needs ~3.4us of PE-busy)
        # during the DMA fill bubble with dependency-free dummy matmuls.
        dummy_w = const_pool.tile([P, P], fp16)
        nc.vector.memset(dummy_w[:], 1.0)
        dummy_x = const_pool.tile([P, 512], fp16)
        nc.vector.memset(dummy_x[:], 1.0)
        dummy_ps = psum_pool.tile([P, 512], fp32, tag="ps", name="dummy_ps")
        for _ in range(6):
            nc.tensor.matmul(dummy_ps[:], dummy_w[:], dummy_x[:],
                             start=True, stop=True)

        # --- fill-phase DMA plan ---
        # Sync queue is dedicated to the PE-critical weight stream (w8 j0 is
        # laddered on scalar, j1+ whole tiles on sync, 4-deep prefetch);
        # everything class-A (xT, wgu16 j0) rides the scalar queue behind the
        # small fp8-phase constants so it never delays fp8 weights.
        w8t0 = w8pool.tile([P, KH // 2, 2, 256], fp8, tag="w8")
        w8t1 = w8pool.tile([P, KH // 2, 2, 256], fp8, tag="w8")
        # x8 ladder on sync; w8gu[0] ladder on scalar (fine steps so the
        # first DR matmuls start ~3us in and never outrun the stream)
        x8_lad = [(0, 1), (1, 2), (2, 4), (4, 7), (7, 12), (12, 20), (20, KH)]
        w8_lad = [(0, 1), (1, 2), (2, 4), (4, 7), (7, 11), (11, KH // 2)]
        for i in range(max(len(x8_lad), len(w8_lad))):
            if i < len(x8_lad):
                xa, xb = x8_lad[i]
                nc.sync.dma_start(x8_sb[:, xa:xb], x8d[:, xa:xb])
            if i < len(w8_lad):
                wa, wb = w8_lad[i]
                nc.scalar.dma_start(w8t0[:, wa:wb], w8gu[0, :, wa:wb])
        nc.scalar.dma_start(sgu8_sb[:], sgu8[:])
        nc.sync.dma_start(w8t1[:, 0:8], w8gu[1, :, 0:8])
        nc.scalar.dma_start(w8t1[:, 8:16], w8gu[1, :, 8:16])
        nc.scalar.dma_start(wcB_sb[:], wcB[:])
        nc.scalar.dma_start(sd8_sb[:], sd8[:])
        wt0 = wpool.tile([P, KH, 256], fp16, tag="wgu")

        # ---- phase 1: gate_up fp8 DoubleRow (class B) ----
        for j in range(NJ):
            if j == 0:
                w8t = w8t0
            elif j == 1:
                w8t = w8t1
            else:
                w8t = w8pool.tile([P, KH // 2, 2, 256], fp8, tag="w8")
                nc.sync.dma_start(w8t[:, 0:8], w8gu[j, :, 0:8])
                nc.scalar.dma_start(w8t[:, 8:16], w8gu[j, :, 8:16])
            psg = psum_pool.tile([P, CB], fp32, tag="ps", name=f"ps8g{j}")
            psu = psum_pool.tile([P, CB], fp32, tag="ps", name=f"ps8u{j}")
            for s in range(KH // 2):
                nc.tensor.matmul(psg[:], w8t[:, s, :, 0:P],
                                 x8_sb[:, 2 * s:2 * s + 2],
                                 start=(s == 0), stop=(s == KH // 2 - 1),
                                 perf_mode=DR)
                nc.tensor.matmul(psu[:], w8t[:, s, :, P:2 * P],
                                 x8_sb[:, 2 * s:2 * s + 2],
                                 start=(s == 0), stop=(s == KH // 2 - 1),
                                 perf_mode=DR)
            # h8 = e4m3( silu(g*sg) * u * su * 0.5 )   (no combine weight here)
            act = tmp_pool.tile([P, CB], fp32, tag="act8")
            nc.scalar.activation(act[:], psg[:],
                                 mybir.ActivationFunctionType.Sigmoid,
                                 scale=sgu8_sb[:, 2 * j:2 * j + 1])
            m1 = tmp_pool.tile([P, CB], fp32, tag="m18")
            nc.vector.tensor_mul(m1[:], act[:], psu[:])
            nc.vector.tensor_mul(m1[:], m1[:], psg[:])
            nc.vector.tensor_scalar_mul(h8_sb[:, j, :], m1[:],
                                        sgu8_sb[:, 2 * j + 1:2 * j + 2])

        # class-A inputs: issued here so they queue BEHIND the gate8 weight
        # stream on scalar (needed only when gate16 starts ~100us in)
        nc.scalar.dma_start(xT_sb[:, 0], xT[0])
        nc.scalar.dma_start(xT_sb[:, 1], xT[1])
        nc.scalar.dma_start(wt0[:], wgu[0])
        nc.scalar.dma_start(sgu_sb[:], sgu[:])
        nc.scalar.dma_start(wcA_sb[:], wcA[:])
        nc.scalar.dma_start(sd_sb[:], sd[:])

        # ---- phase 2: gate_up fp16 (class A), combine weight folded in ----
        for j in range(NJ):
            if j == 0:
                wt = wt0
            else:
                wt = wpool.tile([P, KH, 256], fp16, tag="wgu")
                nc.sync.dma_start(wt[:], wgu[j])
            pss = {t: (psum_pool.tile([P, WA], fp32, tag="ps", name=f"psg{t}"),
                       psum_pool.tile([P, WA], fp32, tag="ps", name=f"psu{t}"))
                   for t in range(tcA)}
            for k in range(KH):
                for t in range(tcA):
                    nc.tensor.matmul(pss[t][0][:], wt[:, k, 0:P],
                                     xT_sb[:, t, k],
                                     start=(k == 0), stop=(k == KH - 1))
                    nc.tensor.matmul(pss[t][1][:], wt[:, k, P:2 * P],
                                     xT_sb[:, t, k],
                                     start=(k == 0), stop=(k == KH - 1))
            for t in range(tcA):
                ts = slice(t * WA, (t + 1) * WA)
                ps_g, ps_u = pss[t]
                # h = sigmoid(g*sg) * g * u * (sg*su) * wcomb
                act = tmp_pool.tile([P, WA], fp32, tag="act")
                nc.scalar.activation(act[:], ps_g[:],
                                     mybir.ActivationFunctionType.Sigmoid,
                                     scale=sgu_sb[:, 2 * j:2 * j + 1])
                m1 = tmp_pool.tile([P, WA], fp32, tag="m1")
                nc.vector.tensor_mul(m1[:], act[:], ps_u[:])
                nc.vector.tensor_mul(m1[:], m1[:], ps_g[:])
                nc.vector.tensor_scalar_mul(m1[:], m1[:],
                                            sgu_sb[:, 2 * j + 1:2 * j + 2])
                nc.vector.tensor_tensor(h_sb[:, j, ts], m1[:], wcA_sb[:, ts],
                                        mybir.AluOpType.mult)

        # ---- phase 3: down fp8 DoubleRow (class B); wd8 prefetches 4-deep
        # during the long gate16 phase so this runs stall-free ----
        for g in range(NG):
            wd8t = wd8pool.tile([P, KI // 2, 2, 256], fp8, tag="wd8")
            nc.sync.dma_start(wd8t[:, 0:4], wd8[g, :, 0:4])
            nc.scalar.dma_start(wd8t[:, 4:7], wd8[g, :, 4:7])
            for half in range(2):
                m = 2 * g + half
                ps = psum_pool.tile([P, CB], fp32, tag="ps", name=f"psd8{m}")
                for s in range(KI // 2):
                    nc.tensor.matmul(ps[:], wd8t[:, s, :, half * P:(half + 1) * P],
                                     h8_sb[:, 2 * s:2 * s + 2],
                                     start=(s == 0), stop=(s == KI // 2 - 1),
                                     perf_mode=DR)
                ot = out_pool.tile([P, CB], fp32, tag="ot8")
                nc.vector.tensor_scalar_mul(ot[:], ps[:], sd8_sb[:, m:m + 1])
                nc.vector.tensor_tensor(ot[:], ot[:], wcB_sb[:],
                                        mybir.AluOpType.mult)
                nc.scalar.dma_start(out[:, m, CA:C], ot[:])

        # ---- phase 4: down fp16 (class A) + per-channel scale ----
        for g in range(NG):
            wdt = wdpool.tile([P, KI, 256], fp16, tag="wd16")
            nc.sync.dma_start(wdt[:, 0:7], wd[g, :, 0:7])
            nc.scalar.dma_start(wdt[:, 7:14], wd[g, :, 7:14])
            for half in range(2):
                m = 2 * g + half
                ot = out_pool.tile([P, CA], fp32, tag="ot")
                for t in range(tcA):
                    ts = slice(t * WA, (t + 1) * WA)
                    ps = psum_pool.tile([P, WA], fp32, tag="ps")
                    for k in range(KI):
                        nc.tensor.matmul(ps[:], wdt[:, k, half * P:(half + 1) * P],
                                         h_sb[:, k, ts],
                                         start=(k == 0), stop=(k == KI - 1))
                    nc.vector.tensor_scalar_mul(ot[:, ts], ps[:], sd_sb[:, m:m + 1])
                    # last groups: drain on the idle sync queue to cut the tail
                    eng = nc.sync if g >= NG - 2 else nc.scalar
                    eng.dma_start(out[:, m, ts], ot[:, ts])

    nc.compile()
    return nc


_NC_CACHE = {}


def _get_nc(CA, CB):
    key = (CA, CB)
    if key not in _NC_CACHE:
        _NC_CACHE[key] = build_moe_nc(CA, CB)
    return _NC_CACHE[key]


def _quant_e4m3(a):
    return np.clip(a, -240.0, 240.0).astype(E4M3)


def _prep_core_inputs(e, CA, CB, hidden, combine, gate_up_w_q, gate_up_scale,
                      down_w_q, down_scale):
    """Device input map for expert e. Class A = CA highest-combine tokens
    (fp16), class B = remainder (fp8). Returns (in_map, idsA, idsB)."""
    WA = CA // 2
    ids = np.nonzero(combine[:, e])[0]
    c = combine[ids, e]
    order = np.argsort(-c, kind="stable")
    nA = min(CA, len(ids))
    idsA = ids[order[:nA]]
    idsB = ids[order[nA:]]
    nB = len(idsB)
    assert nB <= CB, (nB, CB)

    # class A activations [tcA, P, KH, WA] fp16, chunk-major
    xTf = np.zeros((H, CA), np.float16)
    if nA:
        xTf[:, :nA] = hidden[idsA].T.astype(np.float16)
    xT_dev = np.ascontiguousarray(
        xTf.reshape(KH, P, 2, WA).transpose(2, 1, 0, 3))

    # class B activations [P, KH, CB] e4m3
    x8f = np.zeros((H, CB), np.float32)
    if nB:
        x8f[:, :nB] = hidden[idsB].T
    x8_dev = np.ascontiguousarray(
        _quant_e4m3(x8f).reshape(KH, P, CB).transpose(1, 0, 2))

    wgu_c = (gate_up_w_q[e].astype(np.int16) - 128).astype(np.float16)  # [H, 2I]
    wg = wgu_c[:, :I].reshape(H, NJ, P)
    wu = wgu_c[:, I:].reshape(H, NJ, P)
    pairs = np.concatenate([wg, wu], axis=2)                       # [H, NJ, 256]
    wgu_dev = np.ascontiguousarray(
        pairs.reshape(KH, P, NJ, 256).transpose(2, 1, 0, 3))       # [NJ,128,KH,256]
    # fp8 copy with pair-of-k-slabs layout [NJ, P, KH/2, 2, 256]
    w8gu_dev = np.ascontiguousarray(
        pairs.astype(np.float32).reshape(KH // 2, 2, P, NJ, 256)
        .transpose(3, 2, 0, 1, 4).astype(E4M3))

    wd_c = (down_w_q[e].astype(np.int16) - 128).astype(np.float16)  # [I, H]
    wd_dev = np.ascontiguousarray(
        wd_c.reshape(KI, P, NG, 256).transpose(2, 1, 0, 3))        # [NG,128,KI,256]
    wd8_dev = np.ascontiguousarray(
        wd_c.astype(np.float32).reshape(KI // 2, 2, P, NG, 256)
        .transpose(3, 2, 0, 1, 4).astype(E4M3))

    sg = gate_up_scale[e, 0, :I].reshape(NJ, P).astype(np.float32)
    su = gate_up_scale[e, 0, I:].reshape(NJ, P).astype(np.float32)
    sgu_dev = np.empty((P, 2 * NJ), np.float32)
    sgu_dev[:, 0::2] = sg.T
    sgu_dev[:, 1::2] = (sg * su).T
    sgu8_dev = np.empty((P, 2 * NJ), np.float32)
    sgu8_dev[:, 0::2] = sg.T
    sgu8_dev[:, 1::2] = (sg * su * 0.5).T

    sd_dev = np.ascontiguousarray(
        down_scale[e, 0].reshape(H // P, P).T.astype(np.float32))  # [128, 32]
    sd8_dev = np.ascontiguousarray(2.0 * sd_dev)

    wvA = np.zeros(CA, np.float32)
    if nA:
        wvA[:nA] = combine[idsA, e]
    wvB = np.zeros(CB, np.float32)
    if nB:
        wvB[:nB] = combine[idsB, e]
    wcA_dev = np.ascontiguousarray(np.broadcast_to(wvA[None, :], (P, CA)))
    wcB_dev = np.ascontiguousarray(np.broadcast_to(wvB[None, :], (P, CB)))

    return dict(x8=x8_dev, xT=xT_dev, w8gu=w8gu_dev, wgu=wgu_dev,
                wd8=wd8_dev, wd=wd_dev, sgu8=sgu8_dev, sgu=sgu_dev,
                sd8=sd8_dev, sd=sd_dev, wcA=wcA_dev, wcB=wcB_dev), idsA, idsB


def host_routing(expert_affinities, expert_index):
    """Top-k affinity normalization -> dense combine matrix [T, E]."""
    T = expert_index.shape[0]
    sel = np.take_along_axis(expert_affinities.astype(np.float32),
                             expert_index, axis=1)
    sel = sel / sel.sum(axis=1, keepdims=True)
    combine = np.zeros((T, E), np.float32)
    np.add.at(combine,
              (np.repeat(np.arange(T), expert_index.shape[1]),
               expert_index.ravel()),
              sel.ravel())
    return combine


def plan_capacities(combine):
    """(CA, CB) from per-expert routed counts: CB fixed 368-soft, CA covers
    the rest; every expert's lowest-combine (count-CA) pairs go to fp8."""
    counts = (combine > 0).sum(axis=0)
    cmax = int(counts.max())
    cmin = int(counts.min())
    CB = 368
    CA = max(2, cmax - CB)
    CA = min(CA, cmin)          # class A must fill completely on every core
    CA = (CA // 2) * 2
    nBmax = cmax - CA
    CB = int(math.ceil(nBmax / 16)) * 16
    assert CB <= 512
    return CA, CB


def kernel(hidden_states, expert_affinities, gate_up_w_q, gate_up_scale,
           down_w_q, down_scale, expert_index, seq_len=None, **_unused):
    hidden = np.asarray(hidden_states, dtype=np.float32)
    aff = np.asarray(expert_affinities, dtype=np.float32)
    ei = np.asarray(expert_index, dtype=np.int64)
    gq = np.asarray(gate_up_w_q)
    gs = np.asarray(gate_up_scale, dtype=np.float32)
    dq = np.asarray(down_w_q)
    ds = np.asarray(down_scale, dtype=np.float32)
    T = hidden.shape[0]

    combine = host_routing(aff, ei)
    CA, CB = plan_capacities(combine)

    nc = _get_nc(CA, CB)

    in_maps = []
    all_ids = []
    for e in range(E):
        im, idsA, idsB = _prep_core_inputs(e, CA, CB, hidden, combine,
                                           gq, gs, dq, ds)
        in_maps.append(im)
        all_ids.append((idsA, idsB))

    res = run_bass_kernel_spmd(nc, in_maps, list(range(E)))

    y = np.zeros((T, H), np.float32)
    for e in range(E):
        idsA, idsB = all_ids[e]
        out_dev = res.results[e]["out"]            # [128, 32, CA+CB]
        out_full = out_dev.transpose(1, 0, 2).reshape(H, CA + CB)
        if len(idsA):
            y[idsA] += out_full[:, :len(idsA)].T
        if len(idsB):
            y[idsB] += out_full[:, CA:CA + len(idsB)].T
    return y
